# revision 51
# baseline (speedup 1.0000x reference)
"""AttentionFusion Trainium2 kernel: 8-way (batch x sequence) sharded, no collectives.

Reference computation (B=2, N=4096, M=2048, D=256, H=8, dh=32):
    pf   = points @ Wp.T + bp                    [B,N,D]
    q    = (pf @ Wq.T + bq)  -> heads            [B,N,H,dh]
    k    = (vox @ Wk.T + bk) -> heads            [B,M,H,dh]
    v    = (vox @ Wv.T + bv) -> heads            [B,M,H,dh]
    attn = softmax(q @ k.T / sqrt(dh))           [B,H,N,M]
    out  = concat(pf, attn @ v) @ Wf.T + bf      [B,N,D]

Sharding: rows of (B*N) are independent given the batch's voxels, so each of
the 8 cores takes 1024 rows (4 cores per batch) and replicates the cheap k/v
projections for its batch -- no collectives at all.

Device-side design (measured 133.3us/core, +-0.1us, vs 231us naive and
147us first optimized version):
- pf is algebraically eliminated: q = pts @ (Wq@Wp).T + (bq + Wq@bp) and the
  fusion's pf-half facc = pts @ (Wf1@Wp).T + (bf + Wf1@bp) are folded
  host-side into K=3 matmuls straight from the 3-dim points.
- Score matmuls (K=32 per head, bf16) use PE row-tile positions per head
  band; the two matmuls of each item (even/odd voxels) sit on tiles rotated
  by 64 partitions via qT2/kT2 copies built on idle DMA queues.  The PE
  streams ~1 output pixel/cycle total (PSUM write bandwidth), which makes
  the 131K score pixels the kernel's hard floor (~55us); fp8 DoubleRow does
  NOT double-pump on this silicon so bf16 scores are optimal.
- PSUM (8 banks) is 3 score pair-slot tiles (2 banks each) + 2 attended-
  accumulator banks.  Separate tiles: the Tile framework serializes
  accessors of one tile even on disjoint ranges.
- exp(scores) -> fp8 is split ~59/41 between Scalar (table exp) and Vector
  (Schraudolph: round(s*8/ln2 + 56) as int8, bitcast fp8e4m3); both run
  ~1 elem/cycle/lane and sit at ~82% busy -- the second wall.
- exp'd weights feed fp8 DoubleRow attended matmuls (contraction density 256
  per chain step, 8 chained MMs per (head, rc) chain on afb bank rc),
  emitted as pipeline filler; per-head items are rc-major so each chain is
  ready after 8 items.  Drains are deferred past each round's exps.
- The softmax denominator rides the attended matmul as a ones-column in the
  augmented v; divide via approximate-reciprocal (f32-only op) + selection-
  matrix broadcast matmul, dtype hops on the tail-idle Scalar engine.
- Queue placement is load-bearing: output + weight DMAs off the Scalar
  queue, qT2 rotations on Sync, kT2 on GpSimd, k-projection drains
  alternate Scalar/Vector.  ~16us of the runtime is fixed NEFF entry/exit
  preamble (all-engine barriers, engine drains) outside kernel control.

Range contract: softmax weights exp(s) must fit fp8e4 (~[2^-9, 448]) and the
Schraudolph int8 code must stay in [0, 126]: s in [-4.85, 6.06]. The
reference's input distribution gives s in [-3.8, 3.9].
"""

import sys

for _p in ("/root/.axon_site", "/root/.axon_site/_ro/trn_rl_repo",
           "/root/.axon_site/_ro/pypackages", "/opt/trn_rl_repo"):
    if _p not in sys.path:
        sys.path.append(_p)

import numpy as np
import ml_dtypes

BF16 = ml_dtypes.bfloat16
FP8 = ml_dtypes.float8_e4m3

B, N, M, D, H, DH = 2, 4096, 2048, 256, 8, 32
NC = 8                      # cores
R = (B * N) // NC           # 1024 rows per core
CPB = NC // B               # 4 cores per batch
VA = H * (DH + 1)           # 264: v with a ones column per head
MT = M // 128               # 16 voxel tiles

_cached = {}

# Schraudolph exp->fp8e4m3 constants: round(s * 8/ln2 + 56) as int8 == exp(s)
EXP_SCALE = float(8.0 / np.log(2.0))
EXP_BIAS = 56.0


def _build_schedule():
    """Static S-item order: rounds of 3 (h, vt, rc) items. A "heavy" head
    takes 2 items/round, the next head 1, so heads complete in a cascade
    (~every 4 rounds from round 8) and attended work is available as
    pipeline filler throughout."""
    remaining = {h: [(vt, rc) for rc in range(2) for vt in range(MT // 2)]
                 for h in range(H)}
    items = []
    heavy = 0
    while heavy < H:
        light = heavy + 1
        batch = []
        for _ in range(2):
            if remaining[heavy]:
                batch.append((heavy,) + remaining[heavy].pop(0))
        if light < H and remaining[light]:
            batch.append((light,) + remaining[light].pop(0))
        elif remaining[heavy]:
            batch.append((heavy,) + remaining[heavy].pop(0))
        items.extend(batch)
        if not remaining[heavy]:
            heavy += 1
    return items


def _build_nc():
    import concourse.bass as bass
    import concourse.bacc as bacc
    import concourse.tile as tile
    from concourse import mybir

    f32 = mybir.dt.float32
    bf16 = mybir.dt.bfloat16
    fp8 = mybir.dt.float8e4
    i8 = mybir.dt.int8
    Exp = mybir.ActivationFunctionType.Exp
    DR = mybir.MatmulPerfMode.DoubleRow

    nc = bacc.Bacc("TRN2", target_bir_lowering=False, debug=False, num_devices=NC)

    voxT_d = nc.declare_dram_parameter("voxT", [128, 2 * M], fp8, isOutput=False)
    w8_d = nc.declare_dram_parameter("w8", [128, 1568], fp8, isOutput=False)
    small_d = nc.declare_dram_parameter("small8", [8, 1952], bf16, isOutput=False)
    bias_d = nc.declare_dram_parameter("bias_all", [128, 8], f32, isOutput=False)
    out_d = nc.declare_dram_parameter("out", [D, R], f32, isOutput=True)

    items = _build_schedule()
    n_items = len(items)              # 128
    pair_slot = {}                    # (h, vt, rc) -> at8 pair index
    for p, it in enumerate(items):
        pair_slot[it] = p

    # exp engine split: scalar handles ~57% of pairs
    scalar_pairs = set()
    acc = 0.0
    for k in range(n_items):
        acc += 0.594
        if acc >= 1.0:
            scalar_pairs.add(k)
            acc -= 1.0
    # pair -> (engine, local slot) in its engine's staging tile
    eng_slot = {}
    ns = nv = 0
    for p in range(n_items):
        if p in scalar_pairs:
            eng_slot[p] = ("s", ns); ns += 1
        else:
            eng_slot[p] = ("v", nv); nv += 1

    with tile.TileContext(nc) as tc:
        with (
            tc.tile_pool(name="singles", bufs=1) as singles,
            tc.tile_pool(name="state", bufs=1) as state,
            tc.tile_pool(name="stage", bufs=4) as stage_pool,
            tc.tile_pool(name="psum", bufs=1, space="PSUM") as psum,
        ):
            # ---- constants / weights into SBUF ----
            voxT8 = singles.tile([128, 2, M], fp8, tag="voxT8")
            w8 = singles.tile([128, 1568], fp8, tag="w8")
            small8 = singles.tile([8, 1952], bf16, tag="small8")
            bias_sb = singles.tile([128, 8], f32, tag="bias_sb")
            warm = singles.tile([1, 2], f32, tag="warm")
            nc.vector.memset(warm[:], 0.0)
            nc.scalar.activation(warm[0:1, 0:1], warm[0:1, 1:2], Exp)
            nc.sync.dma_start(out=small8[:], in_=small_d[:, :])
            nc.gpsimd.dma_start(out=w8[:], in_=w8_d[:, :])
            for hc in range(2):
                csl = slice(hc * M, (hc + 1) * M)
                nc.sync.dma_start(out=voxT8[:].rearrange("p a b -> p (a b)")[:, csl],
                                  in_=voxT_d[:, csl])
            nc.sync.dma_start(out=bias_sb[:], in_=bias_d[:, :])
            bvrep = singles.tile([128, VA], bf16, tag="bvrep")
            _bv = small_d[0:1, R + 256:R + 256 + VA]
            nc.sync.dma_start(out=bvrep[:], in_=bass.AP(
                tensor=_bv.tensor, offset=_bv.offset, ap=[[0, 128]] + list(_bv.ap[1:])))

            # packed views
            wk8 = w8[:, 0:512].rearrange("p (j c) -> p j c", j=2)
            wv8 = w8[:, 512:1056].rearrange("p (j c) -> p j c", j=2)
            wf28 = w8[:, 1056:1568].rearrange("p (j c) -> p j c", j=2)
            ptsT = small8[0:3, 0:R]
            wqpT = small8[0:3, R:R + 256]
            wfpT = small8[0:3, 1672:1928]
            sel_sb = small8[0:8, R + 520:R + 520 + 256]
            bp_sb = bias_sb[:, 0:2]
            bq_sb = bias_sb[:, 2:4]
            bk_sb = bias_sb[:, 4:6]
            bf_sb = bias_sb[:, 6:8]

            # ---- state tensors ----
            qT = state.tile([128, 2, R], bf16, tag="qT")
            qT2 = state.tile([128, 2, R], bf16, tag="qT2")   # qT rotated 64 partitions
            kT = state.tile([128, 2, M], bf16, tag="kT")
            kT2 = state.tile([128, 2, M], bf16, tag="kT2")   # kT rotated 64 partitions
            vA8 = state.tile([128, MT // 2, 2, 272], fp8, tag="vA8")
            # exp'd scores, slot-major, split per exp engine (avoids bitcast
            # aliasing between the two writers)
            at8s = state.tile([128, 2 * ns, 512], fp8, tag="at8s")
            at8v = state.tile([128, 2 * nv, 512], i8, tag="at8v")
            attT8 = state.tile([128, 2, R], bf16, tag="attT8")   # feature-paired
            attN8 = state.tile([128, 2, R], fp8, tag="attN8")
            facc = state.tile([128, 2, R], bf16, tag="facc")
            denoms = state.tile([8, R], bf16, tag="denoms")
            den32 = state.tile([8, 512], f32, tag="den32")
            recipf = state.tile([8, 512], f32, tag="recipf")
            recipb = state.tile([8, 512], bf16, tag="recipb")

            # ---- psum: 3 score pair-slot tiles (6 banks) + 2 filler/attended
            # banks. Separate tiles: the dep tracker serializes accessors of a
            # single tile even on disjoint ranges.
            scs = [psum.tile([128, 2, 512], f32, tag=f"sc{i}", name=f"sc{i}")
                   for i in range(3)]
            afb = [psum.tile([128, 512], f32, tag=f"afb{i}", name=f"afb{i}")
                   for i in range(2)]

            kTr = [kT[(h % 4) * 32:(h % 4) * 32 + 32, h // 4, :]
                   .rearrange("p (vt k j) -> p vt k j", vt=MT // 2, j=2)
                   for h in range(H)]
            kTr2 = [kT2[((h % 4) * 32 + 64) % 128:((h % 4) * 32 + 64) % 128 + 32, h // 4, :]
                    .rearrange("p (vt k j) -> p vt k j", vt=MT // 2, j=2)
                    for h in range(H)]

            # ---------------- emission helpers ----------------
            def emit_pf_q():
                # q projections: K=3 direct from points (Wq@Wp folded)
                for ft in range(2):
                    for rc in range(2):
                        rsl = slice(rc * 512, (rc + 1) * 512)
                        ps = scs[(ft * 2 + rc) % 3][:, (rc + 1) % 2, :]
                        nc.tensor.matmul(ps, wqpT[:, ft * 128:(ft + 1) * 128],
                                         ptsT[:, rsl], start=True, stop=True)
                        nc.vector.tensor_scalar_add(qT[:, ft, rsl], ps,
                                                    bq_sb[:, ft:ft + 1])
                        nc.sync.dma_start(out=qT2[64:128, ft, rsl], in_=qT[0:64, ft, rsl])
                        nc.sync.dma_start(out=qT2[0:64, ft, rsl], in_=qT[64:128, ft, rsl])

            # k projection chunks: (ft, mc) -> kT[:, ft, mc*512:(mc+1)*512]
            k_count = [0]

            def emit_k(ft, mc, ps):
                msl = slice(mc * 512, (mc + 1) * 512)
                nc.tensor.matmul(ps, wk8[:, :, ft * 128:(ft + 1) * 128],
                                 voxT8[:, :, msl], start=True, stop=True,
                                 perf_mode=DR)
                if k_count[0] % 2 == 0:
                    nc.scalar.activation(kT[:, ft, msl], ps,
                                         mybir.ActivationFunctionType.Identity,
                                         bias=bk_sb[:, ft:ft + 1])
                else:
                    nc.vector.tensor_scalar_add(kT[:, ft, msl], ps,
                                                bk_sb[:, ft:ft + 1])
                k_count[0] += 1
                nc.gpsimd.dma_start(out=kT2[64:128, ft, msl], in_=kT[0:64, ft, msl])
                nc.gpsimd.dma_start(out=kT2[0:64, ft, msl], in_=kT[64:128, ft, msl])

            voxT8r = voxT8[:].rearrange("p a (vt k j) -> p a vt k j", vt=MT // 2, j=2)

            def emit_v(vt, j, ps_full):
                ps = ps_full[:, 0:VA]
                nc.tensor.matmul(ps, voxT8r[:, :, vt, :, j], wv8[:, :, 0:VA],
                                 start=True, stop=True, perf_mode=DR)
                nc.vector.tensor_add(vA8[:, vt, j, 0:VA], ps, bvrep[:])

            def emit_facc(ot, rc, ps):
                osl = slice(ot * 128, (ot + 1) * 128)
                rsl = slice(rc * 512, (rc + 1) * 512)
                nc.tensor.matmul(ps, wfpT[:, osl], ptsT[:, rsl],
                                 start=True, stop=True)
                nc.vector.tensor_scalar_add(facc[:, ot, rsl], ps, bf_sb[:, ot:ot + 1])

            # attended: per-head 16 DR MMs split into 4 units of 4; drain on last
            def emit_att_unit(h, u):
                for k in range(4):
                    i = u * 4 + k
                    rc, vt = i // 8, i % 8
                    ab = afb[rc][0:33, :]
                    p = pair_slot[(h, vt, rc)]
                    eng, q = eng_slot[p]
                    if eng == "s":
                        rhs = at8s[:, 2 * q:2 * q + 2, :]
                    else:
                        rhs = at8v[:, 2 * q:2 * q + 2, :].bitcast(mybir.dt.float8e4)
                    nc.tensor.matmul(ab, vA8[:, vt, :, h * 33:h * 33 + 33], rhs,
                                     start=(vt == 0), stop=(vt == MT // 2 - 1),
                                     perf_mode=DR)
                if u in (1, 3):
                    pending_drains.append((h, u // 2))

            # ---------------- schedule ----------------
            # filler queues ---------------------------------------------------
            filler = []
            # k chunks before first use; order by first item needing them.
            # item (h, vt, rc): weights kTr[h][:, vt, :, j] <- kT[:, h//4, voxels
            # vt*256 .. vt*256+255] -> chunk mc = vt // 2, ft = h // 4
            needed_at = {}
            for idx, (h, vt, rc) in enumerate(items):
                key = ("k", h // 4, vt // 2)
                needed_at.setdefault(key, idx)
            k_order = sorted(needed_at, key=lambda k: needed_at[k])
            filler += [("k", ft, mc) for (_, ft, mc) in k_order]
            filler += [("v", vt, j) for vt in range(MT // 2) for j in range(2)]
            filler += [("facc", ot, rc) for ot in range(2) for rc in range(2)]

            att_units = []            # (h, unit) available
            done_count = {(h, rc): 0 for h in range(H) for rc in range(2)}
            units_done = [0]
            bank_owner = [None, None]  # afb bank -> head mid-chain
            pending_drains = []

            def flush_drains():
                while pending_drains:
                    h, rcd = pending_drains.pop(0)
                    st = stage_pool.tile([33, 512], bf16, tag="st", bufs=2)
                    if h >= 6:
                        nc.scalar.copy(st[:], afb[rcd][0:33, :])
                    else:
                        nc.vector.tensor_copy(st[:], afb[rcd][0:33, :])
                    nc.sync.dma_start(
                        out=attT8[h * 16:(h + 1) * 16, :, rcd * 512:(rcd + 1) * 512],
                        in_=st[0:32, :])
                    nc.sync.dma_start(out=denoms[h:h + 1, rcd * 512:(rcd + 1) * 512],
                                      in_=st[32:33, :])

            def pop_att_unit():
                for idx, (h, u) in enumerate(att_units):
                    bank = 0 if u < 2 else 1
                    starts = u in (0, 2)
                    if starts and bank_owner[bank] is None:
                        bank_owner[bank] = h
                        return att_units.pop(idx)
                    if not starts and bank_owner[bank] == h:
                        bank_owner[bank] = None
                        return att_units.pop(idx)
                return None

            def normalize_rc(rc):
                rsl = slice(rc * 512, (rc + 1) * 512)
                nc.scalar.copy(den32[:], denoms[:, rsl])
                nc.vector.reciprocal_approx_fast(out=recipf[:], in_=den32[:])
                nc.scalar.copy(recipb[:], recipf[:])
                bc = afb[rc][:]
                nc.tensor.matmul(bc, sel_sb[0:8, 0:128],
                                 recipb[0:8, :], start=True, stop=True)
                bcb = bass.AP(tensor=bc.tensor, offset=bc.offset,
                              ap=[list(bc.ap[0]), [0, 2]] + list(bc.ap[1:]))
                nc.vector.tensor_mul(attN8[:, :, rsl], attT8[:, :, rsl], bcb)

            fi = 0

            def emit_proj(op, ps):
                if op[0] == "k":
                    emit_k(op[1], op[2], ps)
                elif op[0] == "v":
                    emit_v(op[1], op[2], ps)
                else:
                    emit_facc(op[1], op[2], ps)

            # prologue: 2 k chunks before q (their drain+rotation chain
            # gates round 0's j=1 scores; their inputs land before q's)
            for _ in range(2):
                op = filler[fi]; fi += 1
                assert op[0] == "k"
                emit_proj(op, afb[fi % 2][:])
            emit_pf_q()
            for _ in range(2):
                op = filler[fi]; fi += 1
                assert op[0] == "k"
                emit_proj(op, afb[fi % 2][:])

            def flush_exp(p, slot3):
                src = scs[slot3][:]
                eng, q = eng_slot[p]
                if eng == "s":
                    nc.scalar.activation(at8s[:, 2 * q:2 * q + 2, :], src, Exp)
                else:
                    nc.vector.tensor_scalar(at8v[:, 2 * q:2 * q + 2, :], src,
                                            EXP_SCALE, EXP_BIAS,
                                            mybir.AluOpType.mult,
                                            mybir.AluOpType.add)

            # main loop: rounds of 3 items -------------------------------------
            n_rounds = (n_items + 2) // 3
            for r in range(n_rounds):
                batch = items[3 * r:3 * r + 3]
                # filler first: its conservative deps point at already-done work
                if fi < len(filler):
                    op = filler[fi]; fi += 1
                    emit_proj(op, afb[fi % 2][:])
                    if fi < len(filler) and r % 2 == 0:
                        op = filler[fi]; fi += 1
                        emit_proj(op, afb[fi % 2][:])
                else:
                    unit = pop_att_unit()
                    if unit is not None:
                        emit_att_unit(*unit)
                        units_done[0] += 1
                        if len(att_units) > 4:
                            unit = pop_att_unit()
                            if unit is not None:
                                emit_att_unit(*unit)
                                units_done[0] += 1
                # score MMs interleaved by j for row-group rotation
                for j in range(2):
                    for bi, (h, vt, rc) in enumerate(batch):
                        hp = (h % 4) * 32
                        slot3 = (3 * r + bi) % 3       # pair-slot 0..2
                        rsl = slice(rc * 512, (rc + 1) * 512)
                        if j == 0:
                            w, q_, pos = kTr[h], qT[hp:hp + 32, h // 4, rsl], hp
                        else:
                            hp2 = (hp + 64) % 128
                            w, q_, pos = kTr2[h], qT2[hp2:hp2 + 32, h // 4, rsl], hp2
                        nc.tensor.matmul(scs[slot3][:, j, :],
                                         w[:, vt, :, j],
                                         q_,
                                         start=True, stop=True,
                                         tile_position=(pos, 0))
                for bi, (h, vt, rc) in enumerate(batch):
                    flush_exp(pair_slot[(h, vt, rc)], (3 * r + bi) % 3)
                    done_count[(h, rc)] += 1
                    if done_count[(h, rc)] == 8:
                        att_units += [(h, 2 * rc), (h, 2 * rc + 1)]
                flush_drains()

            # leftover attended units
            while att_units:
                unit = pop_att_unit()
                if unit is None:
                    unit = att_units.pop(0)
                emit_att_unit(*unit)
                flush_drains()

            # ---- normalize + fusion tail, pipelined per rc chunk ----
            oq = [nc.gpsimd, nc.sync, nc.gpsimd, nc.sync]
            for rc in range(2):
                normalize_rc(rc)
                rsl = slice(rc * 512, (rc + 1) * 512)
                for ot in range(2):
                    osl = slice(ot * 128, (ot + 1) * 128)
                    tps = scs[2 - rc][:, ot, :]
                    nc.tensor.matmul(tps, wf28[:, :, osl], attN8[:, :, rsl],
                                     start=True, stop=True, perf_mode=DR)
                    ob = stage_pool.tile([128, 512], f32, tag="ob", bufs=2)
                    nc.vector.tensor_add(ob[:], tps, facc[:, ot, rsl])
                    oq[rc * 2 + ot].dma_start(out=out_d[osl, rsl], in_=ob[:])

    nc.compile()
    return nc


def _prep_weights(Wp, bp, Wq, bq, Wk, bk, Wv, bv, Wf, bf):
    scale = np.float32(1.0 / np.sqrt(DH))
    WfT = Wf.T
    # pf is folded away: q = pts @ (Wq@Wp).T + (bq + Wq@bp), and the fusion's
    # pf-half is facc = pts @ (Wf1@Wp).T + (bf + Wf1@bp) -- both K=3 matmuls.
    Wqp = (Wq @ Wp) * scale            # [256, 3]
    Wfp = Wf[:, 0:D] @ Wp              # [256, 3]
    bq_f = (bq + Wq @ bp) * scale
    bf_f = bf + Wf[:, 0:D] @ bp

    # fp8 DoubleRow weights: [p, j, c] = W[2p+j, c]
    w8 = np.zeros((128, 1568), dtype=np.float32)
    w8[:, 0:512] = Wk.T.reshape(128, 2 * 256)
    wv_aug = np.zeros((D, VA), dtype=np.float32)
    for h in range(H):
        wv_aug[:, h * 33:h * 33 + 32] = Wv.T[:, h * 32:(h + 1) * 32]
    wv8 = np.zeros((128, 2, 272), dtype=np.float32)
    wv8[:, :, 0:VA] = wv_aug.reshape(128, 2, VA)
    w8[:, 512:1056] = wv8.reshape(128, 544)
    w8[:, 1056:1568] = WfT[256:512, :].reshape(128, 2 * 256)

    # aug row: bv per head + ones column (added via bvrep)
    bvrow = np.zeros((1, VA), dtype=np.float32)
    for h in range(H):
        bvrow[0, h * 33:h * 33 + 32] = bv[h * 32:(h + 1) * 32]
        bvrow[0, h * 33 + 32] = 1.0

    small8 = np.zeros((8, 1952), dtype=np.float32)
    small8[0:3, R:R + 256] = Wqp.T
    small8[0:3, 1672:1928] = Wfp.T
    small8[0:1, R + 256:R + 256 + VA] = bvrow
    for j in range(128):
        small8[j // 16, 1544 + j] = 1.0

    bias_all = np.zeros((128, 8), dtype=np.float32)
    bias_all[:, 2:4] = bq_f.reshape(2, 128).T
    bias_all[:, 4:6] = bk.reshape(2, 128).T
    bias_all[:, 6:8] = bf_f.reshape(2, 128).T

    return {"bias_all": bias_all,
            "w8": w8.astype(FP8)}, small8


def make_in_maps(points, voxel_features, Wp, bp, Wq, bq, Wk, bk, Wv, bv, Wf, bf):
    points = np.asarray(points, dtype=np.float32)
    voxel_features = np.asarray(voxel_features, dtype=np.float32)
    args = [np.asarray(a, dtype=np.float32)
            for a in (Wp, bp, Wq, bq, Wk, bk, Wv, bv, Wf, bf)]
    w, small8 = _prep_weights(*args)
    voxT = [np.ascontiguousarray(voxel_features[b].T).reshape(128, 2 * M).astype(FP8)
            for b in range(B)]
    in_maps = []
    for c in range(NC):
        b, r0 = c // CPB, (c % CPB) * R
        m = dict(w)
        s8 = small8.copy()
        s8[0:3, 0:R] = points[b, r0:r0 + R, :].T
        m["small8"] = s8.astype(BF16)
        m["voxT"] = voxT[b]
        in_maps.append(m)
    return in_maps


def kernel(points, voxel_features, Wp, bp, Wq, bq, Wk, bk, Wv, bv, Wf, bf):
    from concourse.bass_utils import run_bass_kernel_spmd

    if "nc" not in _cached:
        _cached["nc"] = _build_nc()
    nc = _cached["nc"]

    in_maps = make_in_maps(points, voxel_features, Wp, bp, Wq, bq,
                           Wk, bk, Wv, bv, Wf, bf)
    res = run_bass_kernel_spmd(nc, in_maps, core_ids=list(range(NC)), trace=False)

    out = np.empty((B, N, D), dtype=np.float32)
    for c in range(NC):
        b, r0 = c // CPB, (c % CPB) * R
        out[b, r0:r0 + R, :] = res.results[c]["out"].T
    return out



# revision 52
# speedup vs baseline: 1.2154x; 1.2154x over previous
"""AttentionFusion Trainium2 kernel: 8-way (batch x sequence) sharded, no collectives.

Reference computation (B=2, N=4096, M=2048, D=256, H=8, dh=32):
    pf   = points @ Wp.T + bp                    [B,N,D]
    q    = (pf @ Wq.T + bq)  -> heads            [B,N,H,dh]
    k    = (vox @ Wk.T + bk) -> heads            [B,M,H,dh]
    v    = (vox @ Wv.T + bv) -> heads            [B,M,H,dh]
    attn = softmax(q @ k.T / sqrt(dh))           [B,H,N,M]
    out  = concat(pf, attn @ v) @ Wf.T + bf      [B,N,D]

Sharding: rows of (B*N) are independent given the batch's voxels, so each of
the 8 cores takes 1024 rows (4 cores per batch) and replicates the cheap k/v
projections for its batch -- no collectives at all.

Device-side design (measured 133.3us/core, +-0.1us, vs 231us naive and
147us first optimized version):
- pf is algebraically eliminated: q = pts @ (Wq@Wp).T + (bq + Wq@bp) and the
  fusion's pf-half facc = pts @ (Wf1@Wp).T + (bf + Wf1@bp) are folded
  host-side into K=3 matmuls straight from the 3-dim points.
- Score matmuls (K=32 per head, bf16) use PE row-tile positions per head
  band; the two matmuls of each item (even/odd voxels) sit on tiles rotated
  by 64 partitions via qT2/kT2 copies built on idle DMA queues.  The PE
  streams ~1 output pixel/cycle total (PSUM write bandwidth), which makes
  the 131K score pixels the kernel's hard floor (~55us); fp8 DoubleRow does
  NOT double-pump on this silicon so bf16 scores are optimal.
- PSUM (8 banks) is 3 score pair-slot tiles (2 banks each) + 2 attended-
  accumulator banks.  Separate tiles: the Tile framework serializes
  accessors of one tile even on disjoint ranges.
- exp(scores) -> fp8 is split ~59/41 between Scalar (table exp) and Vector
  (Schraudolph: round(s*8/ln2 + 56) as int8, bitcast fp8e4m3); both run
  ~1 elem/cycle/lane and sit at ~82% busy -- the second wall.
- exp'd weights feed fp8 DoubleRow attended matmuls (contraction density 256
  per chain step, 8 chained MMs per (head, rc) chain on afb bank rc),
  emitted as pipeline filler; per-head items are rc-major so each chain is
  ready after 8 items.  Drains are deferred past each round's exps.
- The softmax denominator rides the attended matmul as a ones-column in the
  augmented v; divide via approximate-reciprocal (f32-only op) + selection-
  matrix broadcast matmul, dtype hops on the tail-idle Scalar engine.
- Queue placement is load-bearing: output + weight DMAs off the Scalar
  queue, qT2 rotations on Sync, kT2 on GpSimd, k-projection drains
  alternate Scalar/Vector.  ~16us of the runtime is fixed NEFF entry/exit
  preamble (all-engine barriers, engine drains) outside kernel control.

Range contract: softmax weights exp(s) must fit fp8e4 (~[2^-9, 448]) and the
Schraudolph int8 code must stay in [0, 126]: s in [-4.85, 6.06]. The
reference's input distribution gives s in [-3.8, 3.9].
"""

import sys

for _p in ("/root/.axon_site", "/root/.axon_site/_ro/trn_rl_repo",
           "/root/.axon_site/_ro/pypackages", "/opt/trn_rl_repo"):
    if _p not in sys.path:
        sys.path.append(_p)

import numpy as np
import ml_dtypes

BF16 = ml_dtypes.bfloat16
FP8 = ml_dtypes.float8_e4m3

B, N, M, D, H, DH = 2, 4096, 2048, 256, 8, 32
NC = 8                      # cores
R = (B * N) // NC           # 1024 rows per core
CPB = NC // B               # 4 cores per batch
VA = H * (DH + 1)           # 264: v with a ones column per head
MT = M // 128               # 16 voxel tiles

_cached = {}

# Schraudolph exp->fp8e4m3 constants: round(s * 8/ln2 + 56) as int8 == exp(s)
EXP_SCALE = float(8.0 / np.log(2.0))
EXP_BIAS = 56.0


def _build_schedule():
    """Static S-item order: rounds of 3 (h, vt, rc) items. A "heavy" head
    takes 2 items/round, the next head 1, so heads complete in a cascade
    (~every 4 rounds from round 8) and attended work is available as
    pipeline filler throughout."""
    remaining = {h: [(vt, rc) for rc in range(2) for vt in range(MT // 2)]
                 for h in range(H)}
    items = []
    heavy = 0
    while heavy < H:
        light = heavy + 1
        batch = []
        for _ in range(2):
            if remaining[heavy]:
                batch.append((heavy,) + remaining[heavy].pop(0))
        if light < H and remaining[light]:
            batch.append((light,) + remaining[light].pop(0))
        elif remaining[heavy]:
            batch.append((heavy,) + remaining[heavy].pop(0))
        items.extend(batch)
        if not remaining[heavy]:
            heavy += 1
    return items


def _build_nc():
    import concourse.bass as bass
    import concourse.bacc as bacc
    import concourse.tile as tile
    from concourse import mybir

    f32 = mybir.dt.float32
    bf16 = mybir.dt.bfloat16
    fp8 = mybir.dt.float8e4
    i8 = mybir.dt.int8
    Exp = mybir.ActivationFunctionType.Exp
    DR = mybir.MatmulPerfMode.DoubleRow

    nc = bacc.Bacc("TRN2", target_bir_lowering=False, debug=False, num_devices=NC)

    voxT_d = nc.declare_dram_parameter("voxT", [128, 2 * M], fp8, isOutput=False)
    w8_d = nc.declare_dram_parameter("w8", [128, 1568], fp8, isOutput=False)
    small_d = nc.declare_dram_parameter("small8", [8, 1952], bf16, isOutput=False)
    bias_d = nc.declare_dram_parameter("bias_all", [128, 8], f32, isOutput=False)
    out_d = nc.declare_dram_parameter("out", [D, R], f32, isOutput=True)

    items = _build_schedule()
    n_items = len(items)              # 128
    pair_slot = {}                    # (h, vt, rc) -> at8 pair index
    for p, it in enumerate(items):
        pair_slot[it] = p

    # exp engine split: scalar handles ~57% of pairs
    scalar_pairs = set()
    acc = 0.0
    for k in range(n_items):
        acc += 0.594
        if acc >= 1.0:
            scalar_pairs.add(k)
            acc -= 1.0
    # pair -> (engine, local slot) in its engine's staging tile
    eng_slot = {}
    ns = nv = 0
    for p in range(n_items):
        if p in scalar_pairs:
            eng_slot[p] = ("s", ns); ns += 1
        else:
            eng_slot[p] = ("v", nv); nv += 1

    with tile.TileContext(nc) as tc:
        with (
            tc.tile_pool(name="singles", bufs=1) as singles,
            tc.tile_pool(name="state", bufs=1) as state,
            tc.tile_pool(name="stage", bufs=4) as stage_pool,
            tc.tile_pool(name="psum", bufs=1, space="PSUM") as psum,
        ):
            # ---- constants / weights into SBUF ----
            voxT8 = singles.tile([128, 2, M], fp8, tag="voxT8")
            w8 = singles.tile([128, 1568], fp8, tag="w8")
            small8 = singles.tile([8, 1952], bf16, tag="small8")
            bias_sb = singles.tile([128, 8], f32, tag="bias_sb")
            warm = singles.tile([1, 2], f32, tag="warm")
            nc.vector.memset(warm[:], 0.0)
            nc.scalar.activation(warm[0:1, 0:1], warm[0:1, 1:2], Exp)
            nc.sync.dma_start(out=small8[:], in_=small_d[:, :])
            nc.gpsimd.dma_start(out=w8[:], in_=w8_d[:, :])
            for hc in range(2):
                csl = slice(hc * M, (hc + 1) * M)
                nc.sync.dma_start(out=voxT8[:].rearrange("p a b -> p (a b)")[:, csl],
                                  in_=voxT_d[:, csl])
            nc.sync.dma_start(out=bias_sb[:], in_=bias_d[:, :])
            bvrep = singles.tile([128, VA], bf16, tag="bvrep")
            _bv = small_d[0:1, R + 256:R + 256 + VA]
            nc.sync.dma_start(out=bvrep[:], in_=bass.AP(
                tensor=_bv.tensor, offset=_bv.offset, ap=[[0, 128]] + list(_bv.ap[1:])))

            # packed views
            wk8 = w8[:, 0:512].rearrange("p (j c) -> p j c", j=2)
            wv8 = w8[:, 512:1056].rearrange("p (j c) -> p j c", j=2)
            wf28 = w8[:, 1056:1568].rearrange("p (j c) -> p j c", j=2)
            ptsT = small8[0:3, 0:R]
            wqpT = small8[0:3, R:R + 256]
            wfpT = small8[0:3, 1672:1928]
            sel_sb = small8[0:8, R + 520:R + 520 + 256]
            bp_sb = bias_sb[:, 0:2]
            bq_sb = bias_sb[:, 2:4]
            bk_sb = bias_sb[:, 4:6]
            bf_sb = bias_sb[:, 6:8]

            # ---- state tensors ----
            qT = state.tile([128, 2, R], bf16, tag="qT")
            qT2 = state.tile([128, 2, R], bf16, tag="qT2")   # qT rotated 64 partitions
            kT = state.tile([128, 2, M], bf16, tag="kT")
            kT2 = state.tile([128, 2, M], bf16, tag="kT2")   # kT rotated 64 partitions
            vA8 = state.tile([128, MT // 2, 2, 272], fp8, tag="vA8")
            # exp'd scores, slot-major, split per exp engine (avoids bitcast
            # aliasing between the two writers)
            at8s = state.tile([128, 2 * ns, 512], fp8, tag="at8s")
            at8v = state.tile([128, 2 * nv, 512], i8, tag="at8v")
            attT8 = state.tile([128, 2, R], bf16, tag="attT8")   # feature-paired
            attN8 = state.tile([128, 2, R], fp8, tag="attN8")
            facc = state.tile([128, 2, R], bf16, tag="facc")
            denoms = state.tile([8, R], bf16, tag="denoms")
            den32 = state.tile([8, 512], f32, tag="den32")
            recipf = state.tile([8, 512], f32, tag="recipf")
            recipb = state.tile([8, 512], bf16, tag="recipb")

            # ---- psum: 3 score pair-slot tiles (6 banks) + 2 filler/attended
            # banks. Separate tiles: the dep tracker serializes accessors of a
            # single tile even on disjoint ranges.
            scs = [psum.tile([128, 2, 512], f32, tag=f"sc{i}", name=f"sc{i}")
                   for i in range(3)]
            afb = [psum.tile([128, 512], f32, tag=f"afb{i}", name=f"afb{i}")
                   for i in range(2)]

            kTr = [kT[(h % 4) * 32:(h % 4) * 32 + 32, h // 4, :]
                   .rearrange("p (vt k j) -> p vt k j", vt=MT // 2, j=2)
                   for h in range(H)]
            kTr2 = [kT2[((h % 4) * 32 + 64) % 128:((h % 4) * 32 + 64) % 128 + 32, h // 4, :]
                    .rearrange("p (vt k j) -> p vt k j", vt=MT // 2, j=2)
                    for h in range(H)]

            # ---------------- emission helpers ----------------
            def emit_pf_q():
                # q projections: K=3 direct from points (Wq@Wp folded)
                for ft in range(2):
                    for rc in range(2):
                        rsl = slice(rc * 512, (rc + 1) * 512)
                        ps = scs[(ft * 2 + rc) % 3][:, (rc + 1) % 2, :]
                        nc.tensor.matmul(ps, wqpT[:, ft * 128:(ft + 1) * 128],
                                         ptsT[:, rsl], start=True, stop=True)
                        nc.vector.tensor_scalar_add(qT[:, ft, rsl], ps,
                                                    bq_sb[:, ft:ft + 1])
                        nc.sync.dma_start(out=qT2[64:128, ft, rsl], in_=qT[0:64, ft, rsl])
                        nc.sync.dma_start(out=qT2[0:64, ft, rsl], in_=qT[64:128, ft, rsl])

            # k projection chunks: (ft, mc) -> kT[:, ft, mc*512:(mc+1)*512]
            k_count = [0]

            def emit_k(ft, mc, ps):
                msl = slice(mc * 512, (mc + 1) * 512)
                nc.tensor.matmul(ps, wk8[:, :, ft * 128:(ft + 1) * 128],
                                 voxT8[:, :, msl], start=True, stop=True,
                                 perf_mode=DR)
                if k_count[0] % 2 == 0:
                    nc.scalar.activation(kT[:, ft, msl], ps,
                                         mybir.ActivationFunctionType.Identity,
                                         bias=bk_sb[:, ft:ft + 1])
                else:
                    nc.vector.tensor_scalar_add(kT[:, ft, msl], ps,
                                                bk_sb[:, ft:ft + 1])
                k_count[0] += 1
                nc.gpsimd.dma_start(out=kT2[64:128, ft, msl], in_=kT[0:64, ft, msl])
                nc.gpsimd.dma_start(out=kT2[0:64, ft, msl], in_=kT[64:128, ft, msl])

            voxT8r = voxT8[:].rearrange("p a (vt k j) -> p a vt k j", vt=MT // 2, j=2)

            def emit_v(vt, j, ps_full):
                ps = ps_full[:, 0:VA]
                nc.tensor.matmul(ps, voxT8r[:, :, vt, :, j], wv8[:, :, 0:VA],
                                 start=True, stop=True, perf_mode=DR)
                nc.vector.tensor_add(vA8[:, vt, j, 0:VA], ps, bvrep[:])

            def emit_facc(ot, rc, ps):
                osl = slice(ot * 128, (ot + 1) * 128)
                rsl = slice(rc * 512, (rc + 1) * 512)
                nc.tensor.matmul(ps, wfpT[:, osl], ptsT[:, rsl],
                                 start=True, stop=True)
                nc.vector.tensor_scalar_add(facc[:, ot, rsl], ps, bf_sb[:, ot:ot + 1])

            # attended: per-head 16 DR MMs split into 4 units of 4; drain on last
            def emit_att_unit(h, u):
                for k in range(4):
                    i = u * 4 + k
                    rc, vt = i // 8, i % 8
                    ab = afb[rc][0:33, :]
                    p = pair_slot[(h, vt, rc)]
                    eng, q = eng_slot[p]
                    if eng == "s":
                        rhs = at8s[:, 2 * q:2 * q + 2, :]
                    else:
                        rhs = at8v[:, 2 * q:2 * q + 2, :].bitcast(mybir.dt.float8e4)
                    nc.tensor.matmul(ab, vA8[:, vt, :, h * 33:h * 33 + 33], rhs,
                                     start=(vt == 0), stop=(vt == MT // 2 - 1),
                                     perf_mode=DR)
                if u in (1, 3):
                    pending_drains.append((h, u // 2))

            # ---------------- schedule ----------------
            emit_pf_q()

            # filler queues ---------------------------------------------------
            filler = []
            # k chunks before first use; order by first item needing them.
            # item (h, vt, rc): weights kTr[h][:, vt, :, j] <- kT[:, h//4, voxels
            # vt*256 .. vt*256+255] -> chunk mc = vt // 2, ft = h // 4
            needed_at = {}
            for idx, (h, vt, rc) in enumerate(items):
                key = ("k", h // 4, vt // 2)
                needed_at.setdefault(key, idx)
            k_order = sorted(needed_at, key=lambda k: needed_at[k])
            filler += [("k", ft, mc) for (_, ft, mc) in k_order]
            filler += [("v", vt, j) for vt in range(MT // 2) for j in range(2)]
            filler += [("facc", ot, rc) for ot in range(2) for rc in range(2)]

            att_units = []            # (h, unit) available
            done_count = {(h, rc): 0 for h in range(H) for rc in range(2)}
            units_done = [0]
            bank_owner = [None, None]  # afb bank -> head mid-chain
            pending_drains = []

            def flush_drains():
                while pending_drains:
                    h, rcd = pending_drains.pop(0)
                    st = stage_pool.tile([33, 512], bf16, tag="st", bufs=2)
                    if h >= 6:
                        nc.scalar.copy(st[:], afb[rcd][0:33, :])
                    else:
                        nc.vector.tensor_copy(st[:], afb[rcd][0:33, :])
                    nc.sync.dma_start(
                        out=attT8[h * 16:(h + 1) * 16, :, rcd * 512:(rcd + 1) * 512],
                        in_=st[0:32, :])
                    nc.sync.dma_start(out=denoms[h:h + 1, rcd * 512:(rcd + 1) * 512],
                                      in_=st[32:33, :])

            def pop_att_unit():
                for idx, (h, u) in enumerate(att_units):
                    bank = 0 if u < 2 else 1
                    starts = u in (0, 2)
                    if starts and bank_owner[bank] is None:
                        bank_owner[bank] = h
                        return att_units.pop(idx)
                    if not starts and bank_owner[bank] == h:
                        bank_owner[bank] = None
                        return att_units.pop(idx)
                return None

            def normalize_rc(rc):
                rsl = slice(rc * 512, (rc + 1) * 512)
                nc.scalar.copy(den32[:], denoms[:, rsl])
                nc.vector.reciprocal_approx_fast(out=recipf[:], in_=den32[:])
                nc.scalar.copy(recipb[:], recipf[:])
                bc = afb[rc][:]
                nc.tensor.matmul(bc, sel_sb[0:8, 0:128],
                                 recipb[0:8, :], start=True, stop=True)
                bcb = bass.AP(tensor=bc.tensor, offset=bc.offset,
                              ap=[list(bc.ap[0]), [0, 2]] + list(bc.ap[1:]))
                nc.vector.tensor_mul(attN8[:, :, rsl], attT8[:, :, rsl], bcb)

            fi = 0

            def emit_proj(op, ps):
                if op[0] == "k":
                    emit_k(op[1], op[2], ps)
                elif op[0] == "v":
                    emit_v(op[1], op[2], ps)
                else:
                    emit_facc(op[1], op[2], ps)

            # prologue filler: k chunks for the first items must land first
            for _ in range(4):
                op = filler[fi]; fi += 1
                assert op[0] == "k"
                emit_proj(op, afb[fi % 2][:])

            def flush_exp(p, slot3):
                src = scs[slot3][:]
                eng, q = eng_slot[p]
                if eng == "s":
                    nc.scalar.activation(at8s[:, 2 * q:2 * q + 2, :], src, Exp)
                else:
                    nc.vector.tensor_scalar(at8v[:, 2 * q:2 * q + 2, :], src,
                                            EXP_SCALE, EXP_BIAS,
                                            mybir.AluOpType.mult,
                                            mybir.AluOpType.add)

            # main loop: rounds of 3 items -------------------------------------
            n_rounds = (n_items + 2) // 3
            for r in range(n_rounds):
                batch = items[3 * r:3 * r + 3]
                # filler first: its conservative deps point at already-done work
                if fi < len(filler):
                    op = filler[fi]; fi += 1
                    emit_proj(op, afb[fi % 2][:])
                    if fi < len(filler) and r % 2 == 0:
                        op = filler[fi]; fi += 1
                        emit_proj(op, afb[fi % 2][:])
                else:
                    unit = pop_att_unit()
                    if unit is not None:
                        emit_att_unit(*unit)
                        units_done[0] += 1
                        if len(att_units) > 4:
                            unit = pop_att_unit()
                            if unit is not None:
                                emit_att_unit(*unit)
                                units_done[0] += 1
                # score MMs interleaved by j for row-group rotation
                for j in range(2):
                    for bi, (h, vt, rc) in enumerate(batch):
                        hp = (h % 4) * 32
                        slot3 = (3 * r + bi) % 3       # pair-slot 0..2
                        rsl = slice(rc * 512, (rc + 1) * 512)
                        if j == 0:
                            w, q_, pos = kTr[h], qT[hp:hp + 32, h // 4, rsl], hp
                        else:
                            hp2 = (hp + 64) % 128
                            w, q_, pos = kTr2[h], qT2[hp2:hp2 + 32, h // 4, rsl], hp2
                        nc.tensor.matmul(scs[slot3][:, j, :],
                                         w[:, vt, :, j],
                                         q_,
                                         start=True, stop=True,
                                         tile_position=(pos, 0))
                for bi, (h, vt, rc) in enumerate(batch):
                    flush_exp(pair_slot[(h, vt, rc)], (3 * r + bi) % 3)
                    done_count[(h, rc)] += 1
                    if done_count[(h, rc)] == 8:
                        att_units += [(h, 2 * rc), (h, 2 * rc + 1)]
                flush_drains()

            # leftover attended units
            while att_units:
                unit = pop_att_unit()
                if unit is None:
                    unit = att_units.pop(0)
                emit_att_unit(*unit)
                flush_drains()

            # ---- normalize + fusion tail, pipelined per rc chunk ----
            oq = [nc.gpsimd, nc.sync, nc.gpsimd, nc.sync]
            for rc in range(2):
                normalize_rc(rc)
                rsl = slice(rc * 512, (rc + 1) * 512)
                for ot in range(2):
                    osl = slice(ot * 128, (ot + 1) * 128)
                    tps = scs[2 - rc][:, ot, :]
                    nc.tensor.matmul(tps, wf28[:, :, osl], attN8[:, :, rsl],
                                     start=True, stop=True, perf_mode=DR)
                    ob = stage_pool.tile([128, 512], f32, tag="ob", bufs=2)
                    nc.vector.tensor_add(ob[:], tps, facc[:, ot, rsl])
                    oq[rc * 2 + ot].dma_start(out=out_d[osl, rsl], in_=ob[:])

    nc.compile()
    return nc


def _prep_weights(Wp, bp, Wq, bq, Wk, bk, Wv, bv, Wf, bf):
    scale = np.float32(1.0 / np.sqrt(DH))
    WfT = Wf.T
    # pf is folded away: q = pts @ (Wq@Wp).T + (bq + Wq@bp), and the fusion's
    # pf-half is facc = pts @ (Wf1@Wp).T + (bf + Wf1@bp) -- both K=3 matmuls.
    Wqp = (Wq @ Wp) * scale            # [256, 3]
    Wfp = Wf[:, 0:D] @ Wp              # [256, 3]
    bq_f = (bq + Wq @ bp) * scale
    bf_f = bf + Wf[:, 0:D] @ bp

    # fp8 DoubleRow weights: [p, j, c] = W[2p+j, c]
    w8 = np.zeros((128, 1568), dtype=np.float32)
    w8[:, 0:512] = Wk.T.reshape(128, 2 * 256)
    wv_aug = np.zeros((D, VA), dtype=np.float32)
    for h in range(H):
        wv_aug[:, h * 33:h * 33 + 32] = Wv.T[:, h * 32:(h + 1) * 32]
    wv8 = np.zeros((128, 2, 272), dtype=np.float32)
    wv8[:, :, 0:VA] = wv_aug.reshape(128, 2, VA)
    w8[:, 512:1056] = wv8.reshape(128, 544)
    w8[:, 1056:1568] = WfT[256:512, :].reshape(128, 2 * 256)

    # aug row: bv per head + ones column (added via bvrep)
    bvrow = np.zeros((1, VA), dtype=np.float32)
    for h in range(H):
        bvrow[0, h * 33:h * 33 + 32] = bv[h * 32:(h + 1) * 32]
        bvrow[0, h * 33 + 32] = 1.0

    small8 = np.zeros((8, 1952), dtype=np.float32)
    small8[0:3, R:R + 256] = Wqp.T
    small8[0:3, 1672:1928] = Wfp.T
    small8[0:1, R + 256:R + 256 + VA] = bvrow
    for j in range(128):
        small8[j // 16, 1544 + j] = 1.0

    bias_all = np.zeros((128, 8), dtype=np.float32)
    bias_all[:, 2:4] = bq_f.reshape(2, 128).T
    bias_all[:, 4:6] = bk.reshape(2, 128).T
    bias_all[:, 6:8] = bf_f.reshape(2, 128).T

    return {"bias_all": bias_all,
            "w8": w8.astype(FP8)}, small8


def make_in_maps(points, voxel_features, Wp, bp, Wq, bq, Wk, bk, Wv, bv, Wf, bf):
    points = np.asarray(points, dtype=np.float32)
    voxel_features = np.asarray(voxel_features, dtype=np.float32)
    args = [np.asarray(a, dtype=np.float32)
            for a in (Wp, bp, Wq, bq, Wk, bk, Wv, bv, Wf, bf)]
    w, small8 = _prep_weights(*args)
    voxT = [np.ascontiguousarray(voxel_features[b].T).reshape(128, 2 * M).astype(FP8)
            for b in range(B)]
    in_maps = []
    for c in range(NC):
        b, r0 = c // CPB, (c % CPB) * R
        m = dict(w)
        s8 = small8.copy()
        s8[0:3, 0:R] = points[b, r0:r0 + R, :].T
        m["small8"] = s8.astype(BF16)
        m["voxT"] = voxT[b]
        in_maps.append(m)
    return in_maps


def kernel(points, voxel_features, Wp, bp, Wq, bq, Wk, bk, Wv, bv, Wf, bf):
    from concourse.bass_utils import run_bass_kernel_spmd

    if "nc" not in _cached:
        _cached["nc"] = _build_nc()
    nc = _cached["nc"]

    in_maps = make_in_maps(points, voxel_features, Wp, bp, Wq, bq,
                           Wk, bk, Wv, bv, Wf, bf)
    res = run_bass_kernel_spmd(nc, in_maps, core_ids=list(range(NC)), trace=False)

    out = np.empty((B, N, D), dtype=np.float32)
    for c in range(NC):
        b, r0 = c // CPB, (c % CPB) * R
        out[b, r0:r0 + R, :] = res.results[c]["out"].T
    return out

